# revision 18
# baseline (speedup 1.0000x reference)
"""GNN message-passing layer on 8 TRN2 NeuronCores.

Reference computation (N=16384, D=128):
    a    = adj_mat.astype(f32)            # [N, N]
    deg  = a.sum(axis=0)                  # [N]
    agg  = (a^T @ x) / deg[:, None]       # [N, D]
    out  = relu(agg @ U^T)[None]          # [1, N, D]

Sharding: column-shard adj_mat across the 8 cores (core c owns output
nodes i in [c*2048, (c+1)*2048) and reads adj[:, islice]); x and U are
replicated. The contraction over j (all 16384 rows) is fully local to
each core — no collective — and each core emits its own contiguous
slice of the output.

PE scheme (the kernel is tensor-engine bound): the aggregation runs as
fp8e4 DoubleRow matmuls — 256 contraction rows per pass, 2x the
fp16/bf16 rate.  x is split hi/lo (x = fp8(x) + fp8(x - fp8(x)), ~8
mantissa bits combined) so precision matches fp16, and the two passes
per 256-row double-block cost exactly what one fp16 pass over the same
rows would.  The win over the fp16 baseline: deg rides inside the
weight matrices instead of costing its own matmuls —
  pass A weights = [ones | x_hi[:, 1:]]      -> psum row 0 = deg
  pass B weights = [x_hi[:, 0] | x_lo[:, 1:]] -> psum row 0 = agg_0
Feature 0 therefore gets only single-fp8 precision; its error
contributes ~2.5e-2/sqrt(128) ~ 2e-3 overall, well inside the 2e-2
gate (measured end-to-end ~2.2e-3).  The drain sums A+B on all 128
rows (row 0 of the sum is garbage) and then overwrites row 0 with
B's row 0 — all engine accesses stay 32-partition aligned, which the
BIR verifier requires.

Host-side staging (value-lossless, part of sharding): the adjacency is
row-permuted per 1 MiB chunk so every device DMA is 128 partitions x
8 KiB contiguous (descriptor-cheap), and converted {0,1}->fp8e4 via a
uint8 LUT.  x -> xA/xB fp8 weight tensors in [p, pair, ktile, col]
layout; U -> U^T fp16.
"""

import sys

if "/opt/trn_rl_repo" not in sys.path:
    sys.path.insert(0, "/opt/trn_rl_repo")

import numpy as np

from concourse import bacc, mybir, tile
from concourse.bass import ts
from concourse.bass_utils import run_bass_kernel_spmd

N = 16384  # nodes
D = 128  # features
CORES = 8
S = N // CORES  # 2048 output nodes per core
P = 128  # partitions
JB = N // P  # 128 row-blocks
NPAIR = JB // 2  # 64 DoubleRow double-blocks
IC = S // 512  # 4 psum chunks of 512
T = S // P  # 16 output tiles per core
CH = 8  # row-blocks per adj DMA chunk (2 MB fp8)
NCK = JB // CH  # 16 chunks

F16 = mybir.dt.float16
F32 = mybir.dt.float32
F8 = mybir.dt.float8e4
DR = mybir.MatmulPerfMode.DoubleRow


def build_nc():
    nc = bacc.Bacc("TRN2", target_bir_lowering=False, debug=False)

    a_dram = nc.dram_tensor("a", [N, S], F8, kind="ExternalInput").ap()
    xa_dram = nc.dram_tensor("xa", [P, NPAIR * 2 * D], F8, kind="ExternalInput").ap()
    xb_dram = nc.dram_tensor("xb", [P, NPAIR * 2 * D], F8, kind="ExternalInput").ap()
    ut_dram = nc.dram_tensor("ut", [D, D], F16, kind="ExternalInput").ap()
    # [i_lo, t, e] layout; host un-permutes to [2048, 128]
    out_dram = nc.dram_tensor("out", [P, T * D], F32, kind="ExternalOutput").ap()

    with tile.TileContext(nc) as tc:
        with (
            tc.tile_pool(name="persist", bufs=1) as persist,
            tc.tile_pool(name="adj", bufs=5) as adj_pool,
            tc.tile_pool(name="dram", bufs=1, space="DRAM") as dram_pool,
        ):
            xa = persist.tile([P, NPAIR, 2, D], F8)
            xb = persist.tile([P, NPAIR, 2, D], F8)
            ut16 = persist.tile([D, D], F16)

            ag16 = persist.tile([P, S], F16)
            bsb = persist.tile([P, S], F32)  # psum-B staging (single-PSUM-input rule)
            deg_sb = persist.tile([P, 512], F32)  # rows {0,32,64,96} hold deg
            degT = persist.tile([P, T], F32)
            rdeg = persist.tile([P, T], F32)

            with tc.tile_pool(name="mmps", bufs=1, space="PSUM") as mmps:
                ps_a = [mmps.tile([P, 512], F32, name=f"ps_a{i}") for i in range(IC)]
                ps_b = [mmps.tile([P, 512], F32, name=f"ps_b{i}") for i in range(IC)]

                xa_r = xa_dram.rearrange("p (g r) -> p g r", g=8)
                xb_r = xb_dram.rearrange("p (g r) -> p g r", g=8)
                for ck in range(NCK):
                    af = adj_pool.tile([P, CH, S], F8, tag="af")
                    # alternate the two HWDGE rings; ring 1 (scalar) also
                    # carries the x prologue
                    eng = nc.sync if ck % 2 == 0 else nc.scalar
                    # host layout: chunk ck rows are [p, c, i] with the
                    # per-partition CH*S bytes contiguous
                    src = a_dram[ck * CH * P : (ck + 1) * CH * P, :]
                    src_r = src.rearrange("(p c) i -> p c i", p=P)
                    if ck == 0:
                        # split the first chunk so the opening matmuls are
                        # not gated on a full 2 MB transfer
                        eng.dma_start(af[:, 0:2, :], src_r[:, 0:2, :])
                        eng.dma_start(af[:, 2:4, :], src_r[:, 2:4, :])
                        eng.dma_start(af[:, 4:CH, :], src_r[:, 4:CH, :])
                    else:
                        eng.dma_start(af[:], src_r)
                    # x weights prologue in 512 KB pieces on ring 1: piece g
                    # gates the matmuls from pair 8g; after piece 0, xa and
                    # xb ship one chunk apart to halve the early competition
                    # with the adjacency stream
                    # piece g must land before chunk 2g: xa piece g ships at
                    # even ck=2g-2, xb piece g at odd ck=2g-1
                    if ck == 0:
                        nc.scalar.dma_start(xa[:, ts(0, 8), :, :], xa_r[:, 0, :])
                        nc.scalar.dma_start(xb[:, ts(0, 8), :, :], xb_r[:, 0, :])
                        nc.scalar.dma_start(ut16[:], ut_dram[:])
                        nc.scalar.dma_start(xa[:, ts(1, 8), :, :], xa_r[:, 1, :])
                    elif ck % 2 == 0 and ck <= 12:
                        g = ck // 2 + 1
                        nc.scalar.dma_start(xa[:, ts(g, 8), :, :], xa_r[:, g, :])
                    elif ck % 2 == 1 and ck <= 13:
                        g = (ck + 1) // 2
                        nc.scalar.dma_start(xb[:, ts(g, 8), :, :], xb_r[:, g, :])
                    if ck < NCK - 1:
                        # pair-major: A then B per pair
                        for pp in range(CH // 2):
                            b = ck * (CH // 2) + pp
                            first = b == 0
                            mv = af[:, 2 * pp : 2 * pp + 2, :]
                            for ps, w in ((ps_a, xa), (ps_b, xb)):
                                for ic in range(IC):
                                    nc.tensor.matmul(
                                        ps[ic][:],
                                        w[:, b, :, :],
                                        mv[:, :, ts(ic, 512)],
                                        start=first,
                                        stop=False,
                                        perf_mode=DR,
                                    )
                    else:
                        # last chunk ic-major: each psum chain stops as early
                        # as possible so its drain hides under the remaining
                        # matmul stream
                        for ic in range(IC):
                            for ps, w in ((ps_a, xa), (ps_b, xb)):
                                for pp in range(CH // 2):
                                    b = ck * (CH // 2) + pp
                                    mv = af[:, 2 * pp : 2 * pp + 2, :]
                                    nc.tensor.matmul(
                                        ps[ic][:],
                                        w[:, b, :, :],
                                        mv[:, :, ts(ic, 512)],
                                        start=False,
                                        stop=(pp == CH // 2 - 1),
                                        perf_mode=DR,
                                    )

                # drain, per chunk as its chains stop (the last chunk runs
                # ic-major so chain ic is done 8*(3-ic) matmuls early):
                #   stage B in SBUF (an instruction may read only one PSUM
                #   input), ag16 = A(psum) + B(sbuf) on DVE, deg row out on
                #   DVE, deg store DMA interleaved on scalar.  Row 0 of the
                #   sum is deg + agg_0 garbage; overwrite with B's row 0
                #   (the fp8-single feature-0 agg).
                deg_dram = dram_pool.tile([IC, 512], F32)
                for ic in range(IC):
                    nc.scalar.copy(bsb[:, ts(ic, 512)], ps_b[ic][:])
                    nc.vector.tensor_copy(
                        deg_sb[32 * ic : 32 * ic + 1, :], ps_a[ic][0:1, :]
                    )
                    nc.vector.tensor_tensor(
                        ag16[:, ts(ic, 512)],
                        ps_a[ic][:],
                        bsb[:, ts(ic, 512)],
                        mybir.AluOpType.add,
                    )
                    nc.scalar.dma_start(
                        deg_dram[ic : ic + 1, :], deg_sb[32 * ic : 32 * ic + 1, :]
                    )
                    nc.scalar.copy(ag16[0:1, ts(ic, 512)], bsb[0:1, ts(ic, 512)])

            nc.scalar.dma_start(
                degT[:], deg_dram.rearrange("a b -> (a b)").rearrange("(t p) -> p t", p=P)
            )
            nc.vector.reciprocal(rdeg[:], degT[:])

            # output in quarters so early DMAs overlap the later tiles
            o_q = [persist.tile([P, 4, D], F32, name=f"o_q{q}") for q in range(4)]
            out_r = out_dram.rearrange("p (t d) -> p t d", t=T)
            with tc.tile_pool(name="fps", bufs=3, space="PSUM") as fps:
                for t in range(T):
                    h_ps = fps.tile([P, D], F32, tag="h")
                    nc.tensor.matmul(
                        h_ps[:], ag16[:, ts(t, P)], ut16[:], start=True, stop=True
                    )
                    o_dst = o_q[t // 4][:, t % 4, :]
                    if t % 2 == 0:
                        # ScalarE: out = relu(h * rdeg)
                        nc.scalar.activation(
                            o_dst,
                            h_ps[:],
                            mybir.ActivationFunctionType.Relu,
                            scale=rdeg[:, t : t + 1],
                        )
                    else:
                        # DVE: out = max(h * rdeg, 0)
                        nc.vector.tensor_scalar(
                            o_dst,
                            h_ps[:],
                            rdeg[:, t : t + 1],
                            0.0,
                            mybir.AluOpType.mult,
                            mybir.AluOpType.max,
                        )
                    if t % 4 == 3:
                        q = t // 4
                        eng = nc.sync if q % 2 == 0 else nc.scalar
                        eng.dma_start(out_r[:, ts(q, 4), :], o_q[q][:])

    nc.compile()
    return nc


_NC = None


def _get_nc():
    global _NC
    if _NC is None:
        _NC = build_nc()
    return _NC


# adjacency row permutation: DMA chunk ck wants rows in [p, c] order so
# each partition's CH rows are contiguous in DRAM
def _adj_perm():
    idx = np.arange(N).reshape(NCK, CH, P)  # [ck, c, p]
    return idx.transpose(0, 2, 1).reshape(-1)  # [ck, p, c]


def prep_in_maps(x, adj_mat, U):
    import ml_dtypes

    f8 = ml_dtypes.float8_e4m3
    x = np.asarray(x, dtype=np.float32)
    adj_mat = np.asarray(adj_mat)
    U = np.asarray(U, dtype=np.float32)

    xhi = x.astype(f8).astype(np.float32)
    xlo = (x - xhi).astype(f8).astype(np.float32)
    # weight tensors [P, NPAIR, 2, D]: row j = pair*256 + ktile*128 + p.
    # column 0 carries deg (pass A: ones) and the fp8-single feature 0
    # (pass B); columns 1.. carry hi/lo of features 1..
    wA = np.concatenate([np.ones((N, 1), np.float32), xhi[:, 1:]], axis=1)
    wB = np.concatenate([xhi[:, 0:1], xlo[:, 1:]], axis=1)

    def wfmt(w):
        # [N, D] -> [pair, ktile, p, d] -> [p, pair, ktile, d]
        v = w.reshape(NPAIR, 2, P, D).transpose(2, 0, 1, 3)
        return np.ascontiguousarray(v.astype(f8).reshape(P, NPAIR * 2 * D))

    xa = wfmt(wA)
    xb = wfmt(wB)
    ut = np.ascontiguousarray(U.T.astype(np.float16))

    # adjacency values are {0,1}: exact in fp8e4m3; the int8 bit patterns
    # 0x00/0x38 come from a uint8 LUT (much faster than a float astype
    # over 1 GiB).  Rows are permuted so each DMA chunk is contiguous
    # per partition.
    lut = np.zeros(2, dtype=np.uint8)
    lut[1] = np.array(1.0, dtype=f8).view(np.uint8)
    perm = _adj_perm()
    adj_p = adj_mat[perm]
    in_maps = []
    for c in range(CORES):
        a8 = lut[adj_p[:, c * S : (c + 1) * S]].view(f8)
        in_maps.append({"a": a8, "xa": xa, "xb": xb, "ut": ut})
    return in_maps


def assemble_out(results):
    # per-core out is [128, T*D] in [i_lo, t, e] layout
    parts = []
    for c in range(CORES):
        o = results[c]["out"].reshape(P, T, D).transpose(1, 0, 2).reshape(S, D)
        parts.append(o)
    return np.concatenate(parts, axis=0)[None]


def kernel(x, adj_mat, U, **_):
    nc = _get_nc()
    in_maps = prep_in_maps(x, adj_mat, U)
    res = run_bass_kernel_spmd(nc, in_maps, core_ids=list(range(CORES)))
    return assemble_out(res.results)


# revision 20
# speedup vs baseline: 1.0417x; 1.0417x over previous
"""GNN message-passing layer on 8 TRN2 NeuronCores.

Reference computation (N=16384, D=128):
    a    = adj_mat.astype(f32)            # [N, N]
    deg  = a.sum(axis=0)                  # [N]
    agg  = (a^T @ x) / deg[:, None]       # [N, D]
    out  = relu(agg @ U^T)[None]          # [1, N, D]

Sharding: column-shard adj_mat across the 8 cores (core c owns output
nodes i in [c*2048, (c+1)*2048) and reads adj[:, islice]); x and U are
replicated. The contraction over j (all 16384 rows) is fully local to
each core — no collective — and each core emits its own contiguous
slice of the output.

PE scheme (the kernel is tensor-engine bound): the aggregation runs as
fp8e4 DoubleRow matmuls — 256 contraction rows per pass, 2x the
fp16/bf16 rate.  x is split hi/lo (x = fp8(x) + fp8(x - fp8(x)), ~8
mantissa bits combined) so precision matches fp16, and the two passes
per 256-row double-block cost exactly what one fp16 pass over the same
rows would.  The win over the fp16 baseline: deg rides inside the
weight matrices instead of costing its own matmuls —
  pass A weights = [ones | x_hi[:, 1:]]      -> psum row 0 = deg
  pass B weights = [x_hi[:, 0] | x_lo[:, 1:]] -> psum row 0 = agg_0
Feature 0 therefore gets only single-fp8 precision; its error
contributes ~2.5e-2/sqrt(128) ~ 2e-3 overall, well inside the 2e-2
gate (measured end-to-end ~2.2e-3).  The drain sums A+B on all 128
rows (row 0 of the sum is garbage) and then overwrites row 0 with
B's row 0 — all engine accesses stay 32-partition aligned, which the
BIR verifier requires.

Host-side staging (value-lossless, part of sharding): the adjacency is
row-permuted per 1 MiB chunk so every device DMA is 128 partitions x
8 KiB contiguous (descriptor-cheap), and converted {0,1}->fp8e4 via a
uint8 LUT.  x -> xA/xB fp8 weight tensors in [p, pair, ktile, col]
layout; U -> U^T fp16.
"""

import sys

if "/opt/trn_rl_repo" not in sys.path:
    sys.path.insert(0, "/opt/trn_rl_repo")

import numpy as np

from concourse import bacc, mybir, tile
from concourse.bass import ts
from concourse.bass_utils import run_bass_kernel_spmd

N = 16384  # nodes
D = 128  # features
CORES = 8
S = N // CORES  # 2048 output nodes per core
P = 128  # partitions
JB = N // P  # 128 row-blocks
NPAIR = JB // 2  # 64 DoubleRow double-blocks
IC = S // 512  # 4 psum chunks of 512
T = S // P  # 16 output tiles per core
CH = 8  # row-blocks per adj DMA chunk (2 MB fp8)
NCK = JB // CH  # 16 chunks

F16 = mybir.dt.float16
F32 = mybir.dt.float32
F8 = mybir.dt.float8e4
DR = mybir.MatmulPerfMode.DoubleRow


def build_nc():
    nc = bacc.Bacc("TRN2", target_bir_lowering=False, debug=False)

    a_dram = nc.dram_tensor("a", [N, S], F8, kind="ExternalInput").ap()
    xa_dram = nc.dram_tensor("xa", [P, NPAIR * 2 * D], F8, kind="ExternalInput").ap()
    xb_dram = nc.dram_tensor("xb", [P, NPAIR * 2 * D], F8, kind="ExternalInput").ap()
    ut_dram = nc.dram_tensor("ut", [D, D], F16, kind="ExternalInput").ap()
    # [i_lo, t, e] layout; host un-permutes to [2048, 128]
    out_dram = nc.dram_tensor("out", [P, T * D], F32, kind="ExternalOutput").ap()

    with tile.TileContext(nc) as tc:
        with (
            tc.tile_pool(name="persist", bufs=1) as persist,
            tc.tile_pool(name="adj", bufs=5) as adj_pool,
            tc.tile_pool(name="dram", bufs=1, space="DRAM") as dram_pool,
        ):
            xa = persist.tile([P, NPAIR, 2, D], F8)
            xb = persist.tile([P, NPAIR, 2, D], F8)
            ut16 = persist.tile([D, D], F16)

            ag16 = persist.tile([P, S], F16)
            bsb = persist.tile([P, S], F32)  # psum-B staging (single-PSUM-input rule)
            deg_sb = persist.tile([P, 512], F32)  # rows {0,32,64,96} hold deg
            degT = persist.tile([P, T], F32)
            rdeg = persist.tile([P, T], F32)

            with tc.tile_pool(name="mmps", bufs=1, space="PSUM") as mmps:
                ps_a = [mmps.tile([P, 512], F32, name=f"ps_a{i}") for i in range(IC)]
                ps_b = [mmps.tile([P, 512], F32, name=f"ps_b{i}") for i in range(IC)]

                # x prologue slices: 4 pairs (one chunk's worth) per DMA,
                # shipped one chunk ahead on ring 1
                xa_r = xa_dram.rearrange("p (g r) -> p g r", g=NCK)
                xb_r = xb_dram.rearrange("p (g r) -> p g r", g=NCK)
                for ck in range(NCK):
                    af = adj_pool.tile([P, CH, S], F8, tag="af")
                    # alternate the two HWDGE rings; ring 1 (scalar) also
                    # carries the x prologue
                    eng = nc.sync if ck % 2 == 0 else nc.scalar
                    # host layout: chunk ck rows are [p, c, i] with the
                    # per-partition CH*S bytes contiguous
                    src = a_dram[ck * CH * P : (ck + 1) * CH * P, :]
                    src_r = src.rearrange("(p c) i -> p c i", p=P)
                    if ck == 0:
                        # split the first chunk so the opening matmuls are
                        # not gated on a full 2 MB transfer
                        eng.dma_start(af[:, 0:1, :], src_r[:, 0:1, :])
                        eng.dma_start(af[:, 1:2, :], src_r[:, 1:2, :])
                        eng.dma_start(af[:, 2:4, :], src_r[:, 2:4, :])
                        eng.dma_start(af[:, 4:CH, :], src_r[:, 4:CH, :])
                    else:
                        eng.dma_start(af[:], src_r)
                    if ck == 0:
                        nc.scalar.dma_start(xa[:, ts(0, 4), :, :], xa_r[:, 0, :])
                        nc.scalar.dma_start(xb[:, ts(0, 4), :, :], xb_r[:, 0, :])
                        nc.scalar.dma_start(ut16[:], ut_dram[:])
                    if ck < NCK - 1:
                        g = ck + 1
                        nc.scalar.dma_start(xa[:, ts(g, 4), :, :], xa_r[:, g, :])
                        nc.scalar.dma_start(xb[:, ts(g, 4), :, :], xb_r[:, g, :])
                    if ck < NCK - 1:
                        # pair-major: A then B per pair
                        for pp in range(CH // 2):
                            b = ck * (CH // 2) + pp
                            first = b == 0
                            mv = af[:, 2 * pp : 2 * pp + 2, :]
                            for ps, w in ((ps_a, xa), (ps_b, xb)):
                                for ic in range(IC):
                                    nc.tensor.matmul(
                                        ps[ic][:],
                                        w[:, b, :, :],
                                        mv[:, :, ts(ic, 512)],
                                        start=first,
                                        stop=False,
                                        perf_mode=DR,
                                    )
                    else:
                        # last chunk ic-major: each psum chain stops as early
                        # as possible so its drain hides under the remaining
                        # matmul stream
                        for ic in range(IC):
                            for ps, w in ((ps_a, xa), (ps_b, xb)):
                                for pp in range(CH // 2):
                                    b = ck * (CH // 2) + pp
                                    mv = af[:, 2 * pp : 2 * pp + 2, :]
                                    nc.tensor.matmul(
                                        ps[ic][:],
                                        w[:, b, :, :],
                                        mv[:, :, ts(ic, 512)],
                                        start=False,
                                        stop=(pp == CH // 2 - 1),
                                        perf_mode=DR,
                                    )

                # drain, per chunk as its chains stop (the last chunk runs
                # ic-major so chain ic is done 8*(3-ic) matmuls early):
                #   stage B in SBUF (an instruction may read only one PSUM
                #   input), ag16 = A(psum) + B(sbuf) on DVE, deg row out on
                #   DVE, deg store DMA interleaved on scalar.  Row 0 of the
                #   sum is deg + agg_0 garbage; overwrite with B's row 0
                #   (the fp8-single feature-0 agg).
                deg_dram = dram_pool.tile([IC, 512], F32)
                for ic in range(IC):
                    nc.scalar.copy(bsb[:, ts(ic, 512)], ps_b[ic][:])
                    nc.vector.tensor_copy(
                        deg_sb[32 * ic : 32 * ic + 1, :], ps_a[ic][0:1, :]
                    )
                    nc.vector.tensor_tensor(
                        ag16[:, ts(ic, 512)],
                        ps_a[ic][:],
                        bsb[:, ts(ic, 512)],
                        mybir.AluOpType.add,
                    )
                    # deg bounce rides the otherwise-idle sync queue so the
                    # scalar queue stays free for copies + activations
                    nc.sync.dma_start(
                        deg_dram[ic : ic + 1, :], deg_sb[32 * ic : 32 * ic + 1, :]
                    )
                    nc.scalar.copy(ag16[0:1, ts(ic, 512)], bsb[0:1, ts(ic, 512)])

            nc.sync.dma_start(
                degT[:], deg_dram.rearrange("a b -> (a b)").rearrange("(t p) -> p t", p=P)
            )
            nc.vector.reciprocal(rdeg[:], degT[:])

            # output in quarters so early DMAs overlap the later tiles
            o_q = [persist.tile([P, 4, D], F32, name=f"o_q{q}") for q in range(4)]
            out_r = out_dram.rearrange("p (t d) -> p t d", t=T)
            with tc.tile_pool(name="fps", bufs=3, space="PSUM") as fps:
                for t in range(T):
                    h_ps = fps.tile([P, D], F32, tag="h")
                    nc.tensor.matmul(
                        h_ps[:], ag16[:, ts(t, P)], ut16[:], start=True, stop=True
                    )
                    o_dst = o_q[t // 4][:, t % 4, :]
                    if t % 2 == 0:
                        # ScalarE: out = relu(h * rdeg)
                        nc.scalar.activation(
                            o_dst,
                            h_ps[:],
                            mybir.ActivationFunctionType.Relu,
                            scale=rdeg[:, t : t + 1],
                        )
                    else:
                        # DVE: out = max(h * rdeg, 0)
                        nc.vector.tensor_scalar(
                            o_dst,
                            h_ps[:],
                            rdeg[:, t : t + 1],
                            0.0,
                            mybir.AluOpType.mult,
                            mybir.AluOpType.max,
                        )
                    if t % 4 == 3:
                        q = t // 4
                        eng = nc.sync if q % 2 == 0 else nc.scalar
                        eng.dma_start(out_r[:, ts(q, 4), :], o_q[q][:])

    nc.compile()
    return nc


_NC = None


def _get_nc():
    global _NC
    if _NC is None:
        _NC = build_nc()
    return _NC


# adjacency row permutation: DMA chunk ck wants rows in [p, c] order so
# each partition's CH rows are contiguous in DRAM
def _adj_perm():
    idx = np.arange(N).reshape(NCK, CH, P)  # [ck, c, p]
    return idx.transpose(0, 2, 1).reshape(-1)  # [ck, p, c]


def prep_in_maps(x, adj_mat, U):
    import ml_dtypes

    f8 = ml_dtypes.float8_e4m3
    x = np.asarray(x, dtype=np.float32)
    adj_mat = np.asarray(adj_mat)
    U = np.asarray(U, dtype=np.float32)

    xhi = x.astype(f8).astype(np.float32)
    xlo = (x - xhi).astype(f8).astype(np.float32)
    # weight tensors [P, NPAIR, 2, D]: row j = pair*256 + ktile*128 + p.
    # column 0 carries deg (pass A: ones) and the fp8-single feature 0
    # (pass B); columns 1.. carry hi/lo of features 1..
    wA = np.concatenate([np.ones((N, 1), np.float32), xhi[:, 1:]], axis=1)
    wB = np.concatenate([xhi[:, 0:1], xlo[:, 1:]], axis=1)

    def wfmt(w):
        # [N, D] -> [pair, ktile, p, d] -> [p, pair, ktile, d]
        v = w.reshape(NPAIR, 2, P, D).transpose(2, 0, 1, 3)
        return np.ascontiguousarray(v.astype(f8).reshape(P, NPAIR * 2 * D))

    xa = wfmt(wA)
    xb = wfmt(wB)
    ut = np.ascontiguousarray(U.T.astype(np.float16))

    # adjacency values are {0,1}: exact in fp8e4m3; the int8 bit patterns
    # 0x00/0x38 come from a uint8 LUT (much faster than a float astype
    # over 1 GiB).  Rows are permuted so each DMA chunk is contiguous
    # per partition.
    lut = np.zeros(2, dtype=np.uint8)
    lut[1] = np.array(1.0, dtype=f8).view(np.uint8)
    perm = _adj_perm()
    adj_p = adj_mat[perm]
    in_maps = []
    for c in range(CORES):
        a8 = lut[adj_p[:, c * S : (c + 1) * S]].view(f8)
        in_maps.append({"a": a8, "xa": xa, "xb": xb, "ut": ut})
    return in_maps


def assemble_out(results):
    # per-core out is [128, T*D] in [i_lo, t, e] layout
    parts = []
    for c in range(CORES):
        o = results[c]["out"].reshape(P, T, D).transpose(1, 0, 2).reshape(S, D)
        parts.append(o)
    return np.concatenate(parts, axis=0)[None]


def kernel(x, adj_mat, U, **_):
    nc = _get_nc()
    in_maps = prep_in_maps(x, adj_mat, U)
    res = run_bass_kernel_spmd(nc, in_maps, core_ids=list(range(CORES)))
    return assemble_out(res.results)
